# revision 26
# baseline (speedup 1.0000x reference)
"""Trainium2 Bass kernel for nn_Decoder_755914244448.

Backward-in-time LSTM decoder: B=8192, T=48, F=64, H=128, OUT=1.
Data-parallel over 8 NeuronCores (1024 batch rows per core).

Restructuring (host-side, exact math):
  prev_out_{s} = h_s @ Wd + bd feeds step s+1's input column, so it folds
  into the recurrence:  Wh' = Wh + Wd @ Wx[0:1,:],  b' = b + bd*Wx[0,:].
  Step 0 uses the raw Wh/b plus a K=1 matmul with decoder_init_input.

On-chip layout is gate-major (hidden dim on partitions, batch on the free
axis) so gate biases are per-partition ACT operands and the recurrent
matmul keeps weights stationary.
"""

import os
from contextlib import ExitStack

import numpy as np

os.environ.setdefault("MYCRO_LOCAL_CACHE", "1")

import concourse.bacc as bacc
import concourse.bass as bass
import concourse.mybir as mybir
import concourse.tile as tile

B, T, F, H = 8192, 48, 64, 128
NCORES = 8
BS = B // NCORES          # per-core batch shard
G4 = 4 * H                # 512 gate width
F32 = mybir.dt.float32
F32R = mybir.dt.float32r
F16 = mybir.dt.float16
SIG = mybir.ActivationFunctionType.Sigmoid
TANH = mybir.ActivationFunctionType.Tanh

_cache = {}
last_results = None  # BassKernelResults of the most recent run (for test.py)


def _build_module():
    nc = bacc.Bacc("TRN2", target_bir_lowering=False, debug=False)

    # ---- DRAM I/O ----
    # features fp16 (host-converted): 2-byte dtype allows the xbar
    # transpose-DMA to read gate-major tiles directly from DRAM
    d_feats = nc.dram_tensor("feats16", [BS, T * F], F16,
                             kind="ExternalInput").ap()
    # h0/c0 pre-converted to fp16 on host so the 2-byte xbar transpose-DMA
    # can produce the gate-major layout directly.
    d_h0 = nc.dram_tensor("h016", [BS, H], F16, kind="ExternalInput").ap()
    d_c0 = nc.dram_tensor("c016", [BS, H], F16, kind="ExternalInput").ap()
    # All small fp16 constants packed into ONE tensor (single DMA): the
    # first transpose-DMA may wait on only a few in-flight copies (xbar-mode
    # transition serialization consumes one sync-wait slot per copy).
    # cols 0:512 Wxf(dup 64-row halves) | 512:1024 Wh' | 1024:1536 Wh |
    # 1536:1568 Wd replicated 32x | row0 1568:2080 wx0 | row0 2080:3104 initT
    d_pk16 = nc.dram_tensor("pk16", [128, 3104], F16,
                            kind="ExternalInput").ap()
    # fp32 pack: biasP [128,0:4] | bias0 [128,4:8] | bd broadcast [128,8:9]
    d_pk32 = nc.dram_tensor("pk32", [128, 9], F32, kind="ExternalInput").ap()
    # fp16 output: outT values are already fp16, so the host-side fp32
    # upcast is exact; skipping the on-chip converts shortens the drain
    d_out = nc.dram_tensor("out", [BS, T], F16, kind="ExternalOutput").ap()

    NCHUNK = BS // 128    # 8 batch chunks of 128
    NTP = (T * F) // 128  # 24 transpose blocks (2 time steps each)

    with tile.TileContext(nc) as tc, ExitStack() as ctx:
        const = ctx.enter_context(tc.tile_pool(name="const", bufs=1))
        featT_p = ctx.enter_context(tc.tile_pool(name="featT", bufs=1))
        state_p = ctx.enter_context(tc.tile_pool(name="state", bufs=2))
        gates_p = ctx.enter_context(tc.tile_pool(name="gates", bufs=2))
        stage_p = ctx.enter_context(tc.tile_pool(name="stage", bufs=3))
        z_psum = ctx.enter_context(tc.tile_pool(name="zp", bufs=3, space="PSUM"))
        o_psum = ctx.enter_context(tc.tile_pool(name="op", bufs=1, space="PSUM"))

        # warm the ACT table set (sigmoid+tanh share one) at t=0 so the
        # implicit table load isn't serialized right before the first gate
        warm = const.tile([1, 1], F32, tag="warm")
        nc.vector.memset(warm, 0.0)
        nc.scalar.activation(warm, warm, SIG, bias=0.0, scale=1.0)
        HW2 = BS // 2  # 512: chain width

        # ---- constants / weights (two packed DMAs) ----
        pk16 = const.tile([128, 3104], F16, tag="pk16")
        nc.sync.dma_start(out=pk16, in_=d_pk16)
        pk32 = const.tile([128, 9], F32, tag="pk32")
        nc.sync.dma_start(out=pk32, in_=d_pk32)
        wxf = pk16[:, 0:512]
        whp = pk16[:, 512:1024]
        wh0 = pk16[:, 1024:1536]
        wd32 = pk16[:, 1536:1568]   # Wd x32: out-MMs fill whole col-groups
        wx0 = pk16[0:1, 1568:2080]
        initT = pk16[0:1, 2080:3104]
        biasP = pk32[:, 0:4]
        bias0 = pk32[:, 4:8]
        bdb = pk32[:, 8:9]
        outT = const.tile([T, BS], F16, tag="outT")

        # h0/c0 entry via xbar transpose; c state stays fp32 (accumulator),
        # h fp16 (only consumed by matmuls).  Two independent half-batch
        # "chains" (cols 0:512 / 512:1024) break the loop-carried latency
        # chain: chain B computes while chain A waits on its dependencies.
        featT = [featT_p.tile([128, BS], F16, tag=f"ft{k}", name=f"ft{k}")
                 for k in range(NTP)]

        def transpose_level(k):
            # one xbar DMA per level: in [1024, 128] rows -> out [128, 1024]
            nc.sync.dma_start_transpose(
                featT[k], d_feats[:, k * 128:(k + 1) * 128])

        # featT[23] first: it gates the very first z-matmul; h0/c0 follow
        transpose_level(23)
        hT0, cT0 = [], []
        for x in range(2):
            h0e = state_p.tile([H, HW2], F16, tag=f"h{x}", name=f"h0e{x}")
            nc.sync.dma_start_transpose(h0e, d_h0[x * HW2:(x + 1) * HW2, :])
            hT0.append(h0e)
        for x in range(2):
            c0e = state_p.tile([H, HW2], F16, tag=f"c{x}", name=f"c0e{x}")
            nc.sync.dma_start_transpose(c0e, d_c0[x * HW2:(x + 1) * HW2, :])
            cT0.append(c0e)
        for k in (22, 21):
            transpose_level(k)

        # ---- main recurrence (two interleaved half-batch chains) ----
        hT, cT = list(hT0), list(cT0)
        og = None
        for s in range(T):
            t = T - 1 - s
            toff = 64 * (t % 2)
            ft = featT[t // 2][toff:toff + 64, :]   # [64, BS] f16
            wxm = wxf[toff:toff + 64, :]            # matching base partition
            whx = wh0 if s == 0 else whp
            biasx = bias0 if s == 0 else biasP
            j = s % 4
            if j == 0:
                og = o_psum.tile([128, BS], F32, tag="og")
            # stream remaining transpose levels in one per even step (k=20 at
            # s=0 ... k=0 at s=40); level k is consumed at step 46-2k, so the
            # production lead only grows.
            if s % 2 == 0 and 20 - s // 2 >= 0:
                transpose_level(20 - s // 2)

            def zmm(x, m, ztile):
                """feat + (init) + recurrent matmuls for gate m of chain x."""
                sl = slice(x * HW2, (x + 1) * HW2)
                msl = slice(128 * m, 128 * (m + 1))
                nc.tensor.matmul(ztile, wxm[:, msl], ft[:, sl],
                                 start=True, stop=False)
                if s == 0:
                    nc.tensor.matmul(ztile, wx0[:, msl], initT[:, sl],
                                     start=False, stop=False)
                nc.tensor.matmul(ztile, whx[:, msl], hT[x],
                                 start=False, stop=True)

            gt = [{}, {}]

            def phase1(x):
                # f (zf) FIRST: the slow Pool f*c product gates the c path,
                # so its input must come off ACT one op earlier; then g (zc)
                # and i (zi) for the DVE-side i*g product.
                for m, gname, fn_, dt_ in ((1, "f", SIG, F16),
                                           (2, "g", TANH, F16),
                                           (0, "i", SIG, F16)):
                    zt = z_psum.tile([128, HW2], F32, tag=f"z{x}",
                                     name=f"z{x}_{s}_{m}")
                    zmm(x, m, zt)
                    gv = gates_p.tile([H, HW2], dt_, tag=f"{gname}{x}",
                                      name=f"{gname}{x}_{s}")
                    nc.scalar.activation(gv, zt, fn_, bias=biasx[:, m:m + 1],
                                         scale=1.0)
                    gt[x][gname] = gv
                # fp16 c-state: the loop-carried cN add runs in the DVE's
                # 2x 16-bit mode (593 -> 327 ns on the critical chain)
                t2 = gates_p.tile([H, HW2], F16, tag=f"t2{x}", name=f"t2{x}_{s}")
                nc.gpsimd.tensor_mul(t2, gt[x]["f"], cT[x])
                t1 = gates_p.tile([H, HW2], F16, tag=f"t1{x}", name=f"t1{x}_{s}")
                nc.vector.tensor_mul(t1, gt[x]["i"], gt[x]["g"])
                cN = state_p.tile([H, HW2], F16, tag=f"c{x}", name=f"c{x}_{s}")
                nc.vector.tensor_add(cN, t1, t2)
                cT[x] = cN

            def phase2(x):
                # o gate + tanh(c) + h update + out row (o/tc fp16: o*tc is
                # also chain-critical and gets the 2x DVE mode)
                zt = z_psum.tile([128, HW2], F32, tag=f"z{x}", name=f"zo{x}_{s}")
                zmm(x, 3, zt)
                ov = gates_p.tile([H, HW2], F16, tag=f"o{x}", name=f"o{x}_{s}")
                nc.scalar.activation(ov, zt, SIG, bias=biasx[:, 3:4], scale=1.0)
                tc_t = gates_p.tile([H, HW2], F16, tag=f"tc{x}", name=f"tc{x}_{s}")
                nc.scalar.activation(tc_t, cT[x], TANH, bias=0.0, scale=1.0)
                hN = state_p.tile([H, HW2], F16, tag=f"h{x}", name=f"h{x}_{s}")
                nc.vector.tensor_mul(hN, ov, tc_t)
                hT[x] = hN
                # out rows: Wd replicated over 32 stationary columns fills
                # psum partitions 32j..32j+31 (row 32j is the one consumed;
                # the rest just keep the staging read fully initialized)
                sl = slice(x * HW2, (x + 1) * HW2)
                nc.tensor.matmul(og[32 * j:32 * (j + 1), sl], wd32, hT[x],
                                 start=True, stop=True,
                                 tile_position=(0, 32 * j))

            if s == 0:
                # anti-phase the two chains from the start: chain A runs a
                # full step before chain B begins, so they never contend for
                # the same engine at the same moment
                phase1(0), phase2(0), phase1(1), phase2(1)
            else:
                phase1(0), phase1(1), phase2(0), phase2(1)
            if j == 3:
                gidx = s // 4
                st = stage_p.tile([128, BS], F16, tag="st")
                # full-partition copy: engines can't take partition-strided
                # APs; the unused rows ride along for free on parallel lanes
                nc.vector.tensor_scalar_add(st, og, bdb[:, 0:1])
                # scatter psum-row partitions {0,32,64,96} -> outT rows 4g..4g+3
                r0 = 4 * gidx
                nc.sync.dma_start(out=outT[r0:r0 + 4, :], in_=st[0:128:32, :])

            if s == 41:
                # output columns 0:32 are final after step 31's scatter:
                # ONE batched 3D-AP transpose (each dma_start costs >1us of
                # ring latency), one convert, one 3D-AP DMA out
                o16a = stage_p.tile([128, 256], F16, tag="o16a")
                nc.sync.dma_start_transpose(
                    o16a.rearrange("p (c t) -> p c t", t=32), outT[0:32, :])
                d_out_rA = d_out.rearrange("(c p) t -> p c t", p=128)[:, :, 0:32]
                nc.sync.dma_start(
                    out=d_out_rA,
                    in_=o16a.rearrange("p (c t) -> p c t", t=32))

        # ---- epilogue: outT rows 32:48 -> out[:, 32:48] ----
        # ONE batched 3D-AP transpose, one convert, one 3D-AP DMA
        o16b = stage_p.tile([128, 128], F16, tag="o16b")
        nc.sync.dma_start_transpose(
            o16b.rearrange("p (c t) -> p c t", t=16), outT[32:T, :])
        d_out_r = d_out.rearrange("(c p) t -> p c t", p=128)[:, :, 32:T]
        nc.sync.dma_start(out=d_out_r,
                          in_=o16b.rearrange("p (c t) -> p c t", t=16))

    nc.compile()
    return nc


def _prep_in_maps(inputs):
    feats = np.ascontiguousarray(inputs["decoder_features"], dtype=np.float16)
    init = np.ascontiguousarray(inputs["decoder_init_input"], dtype=np.float32)
    h0 = np.ascontiguousarray(inputs["h0"], dtype=np.float32)
    c0 = np.ascontiguousarray(inputs["c0"], dtype=np.float32)
    Wx = np.asarray(inputs["Wx"], dtype=np.float32)
    Wh = np.asarray(inputs["Wh"], dtype=np.float32)
    b = np.asarray(inputs["b"], dtype=np.float32)
    Wd = np.asarray(inputs["Wd"], dtype=np.float32)
    bd = np.asarray(inputs["bd"], dtype=np.float32)

    wx0 = Wx[0]
    pk16 = np.zeros((128, 3104), np.float16)
    pk16[:, 0:512] = np.vstack([Wx[1:], Wx[1:]])
    pk16[:, 512:1024] = (Wh + Wd @ wx0[None, :]).astype(np.float16)
    pk16[:, 1024:1536] = Wh.astype(np.float16)
    pk16[:, 1536:1568] = np.repeat(Wd.astype(np.float16), 32, axis=1)
    pk16[0, 1568:2080] = wx0.astype(np.float16)
    pk32 = np.zeros((128, 9), np.float32)
    pk32[:, 0:4] = (b + bd[0] * wx0).reshape(4, H).T
    pk32[:, 4:8] = b.reshape(4, H).T
    pk32[:, 8] = bd[0]
    in_maps = []
    for c in range(NCORES):
        sl = slice(c * BS, (c + 1) * BS)
        p16 = pk16.copy()
        p16[0, 2080:3104] = init[sl, 0].astype(np.float16)
        in_maps.append({
            "feats16": feats[sl].reshape(BS, T * F),
            "h016": np.ascontiguousarray(h0[sl], dtype=np.float16),
            "c016": np.ascontiguousarray(c0[sl], dtype=np.float16),
            "pk16": p16,
            "pk32": pk32,
        })
    return in_maps


def kernel(**inputs) -> np.ndarray:
    global last_results
    from concourse.bass_utils import run_bass_kernel_spmd

    if "nc" not in _cache:
        _cache["nc"] = _build_module()
    nc = _cache["nc"]

    in_maps = _prep_in_maps(inputs)
    trace = bool(int(os.environ.get("KERNEL_TRACE", "0")))
    kw = dict(trace=True, trace_cores=[0]) if trace else {}
    try:
        res = run_bass_kernel_spmd(nc, in_maps, core_ids=list(range(NCORES)),
                                   **kw)
    except ModuleNotFoundError:
        # no NTFF profiling hook in this container; run untraced
        res = run_bass_kernel_spmd(nc, in_maps, core_ids=list(range(NCORES)))
    last_results = res
    out = np.concatenate([r["out"] for r in res.results], axis=0)  # [B, T] f16
    return out[..., None].astype(np.float32)


if __name__ == "__main__":
    rng = np.random.default_rng(0)
    fake = {
        "decoder_features": rng.standard_normal((B, T, F), dtype=np.float32),
        "decoder_init_input": rng.standard_normal((B, 1), dtype=np.float32),
        "h0": rng.standard_normal((B, H), dtype=np.float32),
        "c0": rng.standard_normal((B, H), dtype=np.float32),
        "encoder_output": np.zeros((B, 16, F), np.float32),
        "Wx": (rng.standard_normal((F + 1, G4), dtype=np.float32) * 0.05),
        "Wh": (rng.standard_normal((H, G4), dtype=np.float32) * 0.05),
        "b": np.zeros(G4, np.float32),
        "Wd": (rng.standard_normal((H, 1), dtype=np.float32) * 0.05),
        "bd": np.zeros(1, np.float32),
    }
    out = kernel(**fake)
    print("kernel output", out.shape, out.dtype)

